# revision 6
# baseline (speedup 1.0000x reference)
"""Trainium2 Bass kernel for nn_AttentionBlock (pre-LN transformer block), v3.

Reference (per sequence of S=2048, D=2048, H=16 heads, hd=128):
    h  = LN1(x);  q,k,v = h @ W{q,k,v}.T + b
    o  = causal_softmax(q k^T / sqrt(hd)) v
    hp = h + o @ Wo.T + bo
    h2 = LN2(hp); out = h2 + gelu_tanh(h2 @ W1.T + b1) @ W2.T + b2

v3 design over 8 NeuronCores (core c owns flattened rows [512c,512c+512)
and heads {2c, 2c+1}):
  * LN affine (g, b) folded into the weights on the host:
      Wq' = Wq*g1, bq' = bq + Wq@b1  (same k,v);  W1' = W1*g2,
      b1' = b1 + W1@beta2;  bo' = bo + beta1;  b2' = b2 + beta2.
    The kernel only produces standardized z = (x-mu)*rstd, and applies
    g1/g2 as per-partition scales at the two residual adds.
  * LN1 computed ROW-major with one-pass bn_stats/bn_aggr + a single
    fused scalar.activation (scale=rstd, bias=-mu*rstd) per 128-row
    strip, then PE-transposed (bf16) to feature-major zTb.
  * QKV for own rows, all heads (full Wqkv bf16); three 8-way
    AllToAlls (k, q, v) redistribute to head-sharding.
  * attention per (head, row-block) with head-OUTER ordering; as each
    row-block of head hh finishes, its o tile is DMA'd to the o-A2A
    bounce buffer; AllToAll #1 (first heads) fires mid-attention and
    overlaps the second half; A2A #2 fires at the end and is covered
    by the first-heads O-projection matmuls.
  * O-projection is ROW-parallel (full Wo.T, 2MB of A2A traffic
    instead of a 16MB ReduceScatter); accumulates all 16 heads in
    fp32 PSUM, then residual + LN2 + row-parallel MLP.
"""

import numpy as np
from contextlib import ExitStack

import concourse.bass as bass
import concourse.mybir as mybir
import concourse.tile as tile
from concourse.masks import make_identity

AF = mybir.ActivationFunctionType
ALU = mybir.AluOpType
AX = mybir.AxisListType
f32 = mybir.dt.float32
f32r = mybir.dt.float32r
bf16 = mybir.dt.bfloat16

N_CORES = 8
B, S, D, H, HD = 2, 2048, 2048, 16, 128
NROW = B * S              # 4096 flattened rows
ROWS = NROW // N_CORES    # 512 rows per core
NB = NROW // ROWS         # 8 row-blocks (one per core)
NDT = D // 128            # 16 feature tiles of 128
DH = 4 * D                # 8192 MLP hidden
NF1 = DH // 128           # 64 hidden tiles
HL = H // N_CORES         # 2 heads per core
QT = S // ROWS            # 4 q-blocks of 512 per sequence
LN_EPS = 1e-5
INV_SQRT_HD = float(1.0 / np.sqrt(HD))
RUNWAY = 40               # attention QK softmax tiles emitted ahead of AV

REPL_GROUPS = [list(range(N_CORES))]


def build_nc(upto=9, nocc=False):
    nc = bass.Bass("TRN2", target_bir_lowering=False, debug=False,
                   num_devices=N_CORES)

    # ---- kernel I/O ----
    x_in = nc.dram_tensor("x", [ROWS, D], f32, kind="ExternalInput")
    # full Wqkv (LN1 gain folded), columns [Wk.T | Wq.T | Wv.T], head-major
    wqkv_in = nc.dram_tensor("wqkvT", [D, 3 * D], bf16, kind="ExternalInput")
    bqkv_in = nc.dram_tensor("bqkv", [3 * D], f32, kind="ExternalInput")
    wo_in = nc.dram_tensor("woT", [D, D], bf16, kind="ExternalInput")
    bo_in = nc.dram_tensor("bo", [D], f32, kind="ExternalInput")
    w1_in = nc.dram_tensor("w1T", [D, DH], bf16, kind="ExternalInput")
    b1_in = nc.dram_tensor("b1", [DH], f32, kind="ExternalInput")
    w2_in = nc.dram_tensor("w2T", [DH, D], bf16, kind="ExternalInput")
    b2_in = nc.dram_tensor("b2", [D], f32, kind="ExternalInput")
    g1_in = nc.dram_tensor("g1", [D], f32, kind="ExternalInput")
    g2_in = nc.dram_tensor("g2", [D], f32, kind="ExternalInput")
    y_out = nc.dram_tensor("y", [ROWS, D], f32, kind="ExternalOutput")

    # ---- internal DRAM (collective bounce buffers) ----
    a2ak_in = nc.dram_tensor("a2ak_in", [N_CORES, HL * 128, ROWS], bf16)
    a2ak_out = nc.dram_tensor("a2ak_out", [N_CORES, HL * 128, ROWS], bf16)
    a2aq_in = nc.dram_tensor("a2aq_in", [N_CORES, HL * 128, ROWS], bf16)
    a2aq_out = nc.dram_tensor("a2aq_out", [N_CORES, HL * 128, ROWS], bf16)
    a2av_in = nc.dram_tensor("a2av_in", [N_CORES, HL * 128, ROWS], bf16)
    a2av_out = nc.dram_tensor("a2av_out", [N_CORES, HL * 128, ROWS], bf16)
    a2ao = [(nc.dram_tensor(f"a2ao{j}_in", [N_CORES, 128, ROWS], bf16),
             nc.dram_tensor(f"a2ao{j}_out", [N_CORES, 128, ROWS], bf16))
            for j in range(HL)]

    def cc(kind, op, ins, outs):
        if nocc:
            return
        nc.gpsimd.collective_compute(kind, op, replica_groups=REPL_GROUPS,
                                     ins=ins, outs=outs)

    with tile.TileContext(nc) as tc, ExitStack() as top:
        cpool = top.enter_context(tc.tile_pool(name="cst", bufs=1))

        # constants: identities, ones, causal sub-masks
        ident32 = cpool.tile([128, 128], f32, name="ident32")
        make_identity(nc, ident32[:])
        identb = cpool.tile([128, 128], bf16, name="identb")
        nc.vector.tensor_copy(out=identb[:], in_=ident32[:])
        ones128_32 = cpool.tile([128, 1], f32, name="ones128_32")
        nc.vector.memset(ones128_32[:], 1.0)
        ones128r = cpool.tile([128, 1], f32r, name="ones128r")
        nc.vector.tensor_copy(out=ones128r[:], in_=ones128_32[:])
        ones128b = cpool.tile([128, 1], bf16, name="ones128b")
        nc.vector.tensor_copy(out=ones128b[:], in_=ones128_32[:])
        ones1x128_32 = cpool.tile([1, 128], f32, name="ones1x128_32")
        nc.vector.memset(ones1x128_32[:], 1.0)
        ones1x128r = cpool.tile([1, 128], f32r, name="ones1x128r")
        nc.vector.tensor_copy(out=ones1x128r[:], in_=ones1x128_32[:])
        eps_t = cpool.tile([128, 1], f32, name="eps_t")
        nc.vector.memset(eps_t[:], LN_EPS)

        # diagonal causal masks: mask_j[kk, qq] = 1 if kk + 128*j <= qq
        masks = []
        with ExitStack() as mtmp:
            mpool = mtmp.enter_context(tc.tile_pool(name="msk32", bufs=4))
            for j in range(4):
                m32 = mpool.tile([128, 512], f32, name=f"mask32_{j}")
                nc.gpsimd.memset(m32[:], 1.0)
                nc.gpsimd.affine_select(
                    out=m32[:], in_=m32[:], compare_op=ALU.is_ge, fill=0.0,
                    base=-128 * j, channel_multiplier=-1, pattern=[[1, 512]])
                mb = cpool.tile([128, 512], bf16, name=f"maskb_{j}")
                nc.vector.tensor_copy(out=mb[:], in_=m32[:])
                masks.append(mb)

        # per-partition bias/gain columns ([128, n/128], feature = t*128+p)
        def load_cols(name, src, n):
            t = cpool.tile([128, n // 128], f32, name=name)
            nc.sync.dma_start(out=t[:], in_=src.ap().rearrange(
                "(t p) -> p t", p=128))
            return t
        bqkv_c = load_cols("bqkv_c", bqkv_in, 3 * D)    # [128, 48] (k|q|v)
        bo_c = load_cols("bo_c", bo_in, D)              # [128, 16]
        b1_c = load_cols("b1_c", b1_in, DH)             # [128, 64]
        b2_c = load_cols("b2_c", b2_in, D)
        g1_c = load_cols("g1_c", g1_in, D)
        g2_c = load_cols("g2_c", g2_in, D)

        wqkv_src = wqkv_in.ap().rearrange("(t p) f -> p t f", p=128)
        wo_src = wo_in.ap().rearrange("(h p) f -> p h f", p=128)
        w1src = w1_in.ap().rearrange("(t p) f -> p t f", p=128)
        w2src = w2_in.ap().rearrange("(t p) f -> p t f", p=128)

        # long-lived pools (LIFO close order: h2b < zb < att)
        h2b_es = ExitStack()
        h2b_pool = h2b_es.enter_context(tc.tile_pool(name="h2bp", bufs=1))
        zb_es = ExitStack()
        zb_pool = zb_es.enter_context(tc.tile_pool(name="zbp", bufs=1))
        zTb = zb_pool.tile([128, NDT, 512], bf16, name="zTb")

        # ============ Phase 1: row-major LN1 -> transpose -> zTb ==========
        with ExitStack() as ph1:
            xrows = ph1.enter_context(tc.tile_pool(name="xrows", bufs=2))
            lnw = ph1.enter_context(tc.tile_pool(name="lnw", bufs=2))
            ps_tp = ph1.enter_context(tc.tile_pool(name="ps_tp", bufs=4,
                                                   space="PSUM"))
            for st in range(4):
                xrow = xrows.tile([128, D], f32, name="xrow")
                nc.sync.dma_start(out=xrow[:],
                                  in_=x_in[st * 128:(st + 1) * 128, :])
                stats = lnw.tile([128, 4, 6], f32, name="stats")
                for sg in range(4):
                    nc.vector.bn_stats(out=stats[:, sg, :],
                                       in_=xrow[:, sg * 512:(sg + 1) * 512])
                mv = lnw.tile([128, 2], f32, name="mv")
                nc.vector.bn_aggr(out=mv[:], in_=stats[:])
                sq = lnw.tile([128, 1], f32, name="sq")
                nc.scalar.activation(out=sq[:], in_=mv[:, 1:2], func=AF.Sqrt,
                                     bias=eps_t[:], scale=1.0)
                rstd = lnw.tile([128, 1], f32, name="rstd")
                nc.vector.reciprocal(out=rstd[:], in_=sq[:])
                mr = lnw.tile([128, 1], f32, name="mr")
                nc.vector.tensor_mul(out=mr[:], in0=mv[:, 0:1], in1=rstd[:])
                nmr = lnw.tile([128, 1], f32, name="nmr")
                nc.scalar.activation(out=nmr[:], in_=mr[:], func=AF.Copy,
                                     scale=-1.0)
                zrow = lnw.tile([128, D], bf16, name="zrow")
                nc.scalar.activation(out=zrow[:], in_=xrow[:],
                                     func=AF.Identity, bias=nmr[:],
                                     scale=rstd[:])
                for tq in range(4):
                    tp = ps_tp.tile([128, 512], bf16, name="tp")
                    for i in range(4):
                        t = tq * 4 + i
                        nc.tensor.transpose(
                            tp[:, i * 128:(i + 1) * 128],
                            zrow[:, t * 128:(t + 1) * 128], identb[:])
                    nc.vector.tensor_copy(
                        out=zTb[:, tq * 4:(tq + 1) * 4,
                                st * 128:(st + 1) * 128],
                        in_=tp[:].rearrange("p (i r) -> p i r", i=4))

        # ============ Phase 2: local QKV (all heads, own rows) ============
        # 12 groups of 4 ftiles; k = groups 0-3, q = 4-7, v = 8-11
        with ExitStack() as ph2:
            wq = ph2.enter_context(tc.tile_pool(name="wq", bufs=3))
            kqv = ph2.enter_context(tc.tile_pool(name="kqv", bufs=1))
            ps_qkv = ph2.enter_context(tc.tile_pool(name="ps_qkv",
                                                    bufs=2, space="PSUM"))
            locs = [kqv.tile([128, NDT, 512], bf16, name=nm)
                    for nm in ("k_loc", "q_loc", "v_loc")]
            for g in range(12):
                wch = wq.tile([128, NDT, 512], bf16, name="wch")
                nc.sync.dma_start(
                    out=wch[:],
                    in_=wqkv_src[:, :, g * 512:(g + 1) * 512])
                accs = [ps_qkv.tile([128, 512], f32, name=f"qa{ff}")
                        for ff in range(4)]
                for t in range(NDT):
                    for ff in range(4):
                        nc.tensor.matmul(
                            accs[ff][:],
                            wch[:, t, ff * 128:(ff + 1) * 128],
                            zTb[:, t, :],
                            start=(t == 0), stop=(t == NDT - 1))
                dst = locs[g // 4]
                for ff in range(4):
                    ftg = g * 4 + ff
                    nc.scalar.activation(
                        out=dst[:, ftg % 16, :], in_=accs[ff][:],
                        func=AF.Identity,
                        bias=bqkv_c[:, ftg:ftg + 1], scale=1.0)
                if g % 4 == 3:
                    which = g // 4
                    buf_in = (a2ak_in, a2aq_in, a2av_in)[which]
                    buf_out = (a2ak_out, a2aq_out, a2av_out)[which]
                    nc.sync.dma_start(
                        out=buf_in.ap().rearrange(
                            "c (u p) s -> p (c u) s", p=128),
                        in_=locs[which][:])
                    cc("AllToAll", ALU.bypass, ins=[buf_in.ap()],
                       outs=[buf_out.ap()])

        if upto < 2:
            _early_out(nc, tc, y_out)
            zb_es.close()
            h2b_es.close()
            _split_multiwaits(nc)
            return nc

        # ============ Phase 3: attention =================================
        orx_es = ExitStack()
        orxp = orx_es.enter_context(tc.tile_pool(name="orxp", bufs=1))
        o_rx = [orxp.tile([128, NB, 512], bf16, name=f"o_rx{j}")
                for j in range(HL)]
        att_es = ExitStack()
        attp = att_es.enter_context(tc.tile_pool(name="attp", bufs=1))
        kT = attp.tile([128, HL, NB, 512], bf16, name="kT")
        qT = attp.tile([128, HL, NB, 512], bf16, name="qT")
        vT = attp.tile([128, HL, NB, 512], bf16, name="vT")
        oT = attp.tile([128, HL, NB, 512], bf16, name="oT")
        ksrc = (a2ak_in if nocc else a2ak_out).ap().rearrange(
            "c (hh p) s -> p hh c s", p=128)
        qsrc = (a2aq_in if nocc else a2aq_out).ap().rearrange(
            "c (hh p) s -> p hh c s", p=128)
        vsrc = (a2av_in if nocc else a2av_out).ap().rearrange(
            "c (hh p) s -> p hh c s", p=128)
        for hh in range(HL):
            nc.sync.dma_start(out=kT[:, hh, :, :], in_=ksrc[:, hh, :, :])
        for hh in range(HL):
            nc.sync.dma_start(out=qT[:, hh, :, :], in_=qsrc[:, hh, :, :])
        for hh in range(HL):
            nc.sync.dma_start(out=vT[:, hh, :, :], in_=vsrc[:, hh, :, :])

        if upto < 3:
            _early_out(nc, tc, y_out)
            att_es.close()
            orx_es.close()
            zb_es.close()
            h2b_es.close()
            _split_multiwaits(nc)
            return nc

        with ExitStack() as ph4:
            aw = ph4.enter_context(tc.tile_pool(name="aw", bufs=RUNWAY + 3))
            fin = ph4.enter_context(tc.tile_pool(name="fin", bufs=2))
            ps_lg = ph4.enter_context(tc.tile_pool(name="ps_lg",
                                                   bufs=3, space="PSUM"))
            ps_oa = ph4.enter_context(tc.tile_pool(name="ps_oa",
                                                   bufs=2, space="PSUM"))
            ps_sm = ph4.enter_context(tc.tile_pool(name="ps_sm",
                                                   bufs=1, space="PSUM"))
            ps_rb = ph4.enter_context(tc.tile_pool(name="ps_rb",
                                                   bufs=1, space="PSUM"))
            # head-OUTER step order: all blocks of local head 0, then head 1
            steps = []
            for hh in range(HL):
                for b in range(NB):
                    seq, qb = b // QT, b % QT
                    nkt = 4 * (qb + 1)
                    for kt in range(nkt):
                        steps.append((b, seq, qb, hh, kt, nkt))

            p_tiles = {}
            accs = {}

            def emit_qk(i):
                b, seq, qb, hh, kt, nkt = steps[i]
                qcol = qT[:, hh, b, :]
                kcol = kT[:, hh, seq * QT + kt // 4,
                          (kt % 4) * 128:(kt % 4) * 128 + 128]
                lg = ps_lg.tile([128, 512], f32, name="lg")
                nc.tensor.matmul(lg[:], kcol, qcol, start=True, stop=True)
                p = aw.tile([128, 512], bf16, name="p")
                nc.scalar.activation(out=p[:], in_=lg[:], func=AF.Exp,
                                     scale=INV_SQRT_HD)
                j = kt - 4 * qb
                if j >= 0:
                    nc.vector.tensor_mul(out=p[:], in0=p[:],
                                         in1=masks[j][:])
                p_tiles[i] = p

            def emit_av(i):
                b, seq, qb, hh, kt, nkt = steps[i]
                if kt == 0:
                    accs[(b, hh)] = (
                        ps_oa.tile([128, 512], f32, name="oacc"),
                        ps_sm.tile([1, 512], f32, name="sacc"))
                oacc, sacc = accs[(b, hh)]
                p = p_tiles.pop(i)
                vrow = vT[:, hh, seq * QT + kt // 4,
                          (kt % 4) * 128:(kt % 4) * 128 + 128]
                nc.tensor.matmul(oacc[:], vrow, p[:],
                                 start=(kt == 0), stop=(kt == nkt - 1))
                nc.tensor.matmul(sacc[:], ones128b[:], p[:],
                                 start=(kt == 0), stop=(kt == nkt - 1))
                if kt == nkt - 1:
                    recip32 = fin.tile([1, 512], f32, name="recip32")
                    nc.vector.reciprocal(out=recip32[:], in_=sacc[:])
                    recip = fin.tile([1, 512], f32r, name="recip")
                    nc.scalar.activation(out=recip[:], in_=recip32[:],
                                         func=AF.Copy, scale=1.0)
                    rb = ps_rb.tile([128, 512], f32, name="rb")
                    nc.tensor.matmul(rb[:], ones1x128r[:],
                                     recip[:], start=True, stop=True)
                    ocol = oT[:, hh, b, :]
                    nc.scalar.activation(out=ocol, in_=oacc[:],
                                         func=AF.Copy, scale=1.0)
                    nc.vector.tensor_mul(out=ocol, in0=ocol,
                                         in1=rb[:])
                    # ship this block's o to the A2A bounce buffer
                    nc.sync.dma_start(
                        out=a2ao[hh][0].ap().rearrange(
                            "c p s -> p c s")[:, b, :],
                        in_=ocol)

            nq = min(RUNWAY, len(steps))
            for jj in range(nq):
                emit_qk(jj)
            # in-place transpose of vT -> v, after the QK runway so the
            # in-order PE queue is not blocked on A2A-v before it.
            with ExitStack() as tp3s:
                ps_tp2 = tp3s.enter_context(tc.tile_pool(
                    name="ps_tp2", bufs=1, space="PSUM"))
                for hh in range(HL):
                    for qb_ in range(NB):
                        tp = ps_tp2.tile([128, 512], bf16, name="tpv")
                        for sub in range(4):
                            off = sub * 128
                            nc.tensor.transpose(
                                tp[:, off:off + 128],
                                vT[:, hh, qb_, off:off + 128],
                                identb[:])
                        nc.vector.tensor_copy(out=vT[:, hh, qb_, :],
                                              in_=tp[:])
            for i in range(len(steps)):
                if i + RUNWAY < len(steps):
                    emit_qk(i + RUNWAY)
                emit_av(i)
                b, seq, qb, hh, kt, nkt = steps[i]
                if b == NB - 1 and kt == nkt - 1:
                    # all blocks of local head hh done -> fire o-A2A(hh)
                    cc("AllToAll", ALU.bypass, ins=[a2ao[hh][0].ap()],
                       outs=[a2ao[hh][1].ap()])
                    nc.sync.dma_start(
                        out=o_rx[hh][:],
                        in_=(a2ao[hh][0] if nocc
                             else a2ao[hh][1]).ap().rearrange(
                            "c p s -> p c s"))

        att_es.close()

        if upto < 4:
            _early_out(nc, tc, y_out)
            orx_es.close()
            zb_es.close()
            h2b_es.close()
            _split_multiwaits(nc)
            return nc

        # ==== Phase 4: O-projection (all 16 heads) + residual + LN2 ======
        h2Tb = h2b_pool.tile([128, NDT, 512], bf16, name="h2Tb")
        with ExitStack() as ph5:
            # full Wo.T [in-feature(head-major), out-feature]; even heads
            # (o_rx[0] sources) first so the first O-proj matmuls start
            # after a 4MB DMA, covering the tail of o-A2A #2
            wosp = ph5.enter_context(tc.tile_pool(name="wosp", bufs=1))
            wos = wosp.tile([128, H, D], bf16, name="wos")
            nc.sync.dma_start(out=wos[:, 0::2, :], in_=wo_src[:, 0::2, :])
            nc.sync.dma_start(out=wos[:, 1::2, :], in_=wo_src[:, 1::2, :])
            hp_pool = ph5.enter_context(tc.tile_pool(name="hpp", bufs=1))
            ow = ph5.enter_context(tc.tile_pool(name="ow", bufs=3))
            ps_op = ph5.enter_context(tc.tile_pool(name="ps_op", bufs=2,
                                                   space="PSUM"))
            ps_st = ph5.enter_context(tc.tile_pool(name="ps_st", bufs=1,
                                                   space="PSUM"))
            hpost = hp_pool.tile([128, NDT, 512], f32r, name="hpost")
            s1 = ps_st.tile([1, 512], f32, name="ln2_s1")
            s2 = ps_st.tile([1, 512], f32, name="ln2_s2")
            # head order: o_rx[0] heads (2c) first, then o_rx[1] (2c+1),
            # so the first 8 matmuls only need A2A#1 data
            for dt in range(NDT):
                op = ps_op.tile([128, 512], f32, name="op")
                for j in range(H):
                    hh, c = j // NB, j % NB
                    h = 2 * c + hh
                    nc.tensor.matmul(
                        op[:], wos[:, h, dt * 128:(dt + 1) * 128],
                        o_rx[hh][:, c, :],
                        start=(j == 0), stop=(j == H - 1))
                opb = ow.tile([128, 512], f32, name="opb")
                nc.scalar.activation(out=opb[:], in_=op[:],
                                     func=AF.Identity,
                                     bias=bo_c[:, dt:dt + 1], scale=1.0)
                # hp = z*g1 + (oproj + bo')
                nc.vector.scalar_tensor_tensor(
                    out=hpost[:, dt, :], in0=zTb[:, dt, :],
                    scalar=g1_c[:, dt:dt + 1], in1=opb[:],
                    op0=ALU.mult, op1=ALU.add)
                # LN2 stats
                nc.tensor.matmul(s1[:1, :], ones128r[:], hpost[:, dt, :],
                                 start=(dt == 0), stop=(dt == NDT - 1))
                hsq = ow.tile([128, 512], f32r, name="hsq")
                nc.scalar.activation(out=hsq[:], in_=hpost[:, dt, :]
                                     .bitcast(f32), func=AF.Square,
                                     scale=1.0)
                nc.tensor.matmul(s2[:1, :], ones128r[:], hsq[:],
                                 start=(dt == 0), stop=(dt == NDT - 1))

            # LN2 normalize (no affine): z2 = (hp - mu) * rstd
            one = ph5.enter_context(tc.tile_pool(name="ln2o", bufs=1))
            ps_bc = ph5.enter_context(tc.tile_pool(name="ln2pb", bufs=2,
                                                   space="PSUM"))
            mean = one.tile([1, 512], f32, name="ln2_mean")
            var = one.tile([1, 512], f32, name="ln2_var")
            rstd32 = one.tile([1, 512], f32, name="ln2_rstd32")
            rstd = one.tile([1, 512], f32r, name="ln2_rstd")
            msc32 = one.tile([1, 512], f32, name="ln2_msc32")
            mscr = one.tile([1, 512], f32r, name="ln2_mscr")
            nc.scalar.activation(out=mean[:], in_=s1[:], func=AF.Copy,
                                 scale=1.0 / D)
            nc.vector.tensor_mul(out=var[:], in0=mean[:], in1=mean[:])
            nc.vector.scalar_tensor_tensor(out=var[:], in0=s2[:],
                                           scalar=1.0 / D, in1=var[:],
                                           op0=ALU.mult, op1=ALU.subtract)
            nc.scalar.activation(out=rstd32[:], in_=var[:], func=AF.Sqrt,
                                 bias=eps_t[:1, :], scale=1.0)
            nc.vector.reciprocal(out=rstd32[:], in_=rstd32[:])
            nc.scalar.activation(out=rstd[:], in_=rstd32[:], func=AF.Copy,
                                 scale=1.0)
            nc.vector.tensor_mul(out=msc32[:], in0=mean[:], in1=rstd32[:])
            nc.scalar.activation(out=mscr[:], in_=msc32[:], func=AF.Copy,
                                 scale=-1.0)
            bc1 = ps_bc.tile([128, 512], f32, name="ln2_bc1")
            bc2 = ps_bc.tile([128, 512], f32, name="ln2_bc2")
            nc.tensor.matmul(bc1[:], ones1x128r[:], rstd[:], start=True,
                             stop=True)
            nc.tensor.matmul(bc2[:], ones1x128r[:], mscr[:], start=True,
                             stop=True)
            bc1s = one.tile([128, 512], f32, name="ln2_bc1s")
            bc2s = one.tile([128, 512], f32, name="ln2_bc2s")
            nc.scalar.activation(out=bc1s[:], in_=bc1[:], func=AF.Copy,
                                 scale=1.0)
            nc.scalar.activation(out=bc2s[:], in_=bc2[:], func=AF.Copy,
                                 scale=1.0)
            for dt in range(NDT):
                tmp = ow.tile([128, 512], f32, name="ln2_tmp")
                if dt % 2 == 0:
                    nc.vector.tensor_mul(out=tmp[:],
                                         in0=hpost[:, dt, :].bitcast(f32),
                                         in1=bc1s[:])
                    nc.vector.tensor_add(out=h2Tb[:, dt, :], in0=tmp[:],
                                         in1=bc2s[:])
                else:
                    nc.gpsimd.tensor_mul(out=tmp[:],
                                         in0=hpost[:, dt, :].bitcast(f32),
                                         in1=bc1s[:])
                    nc.gpsimd.tensor_add(out=h2Tb[:, dt, :], in0=tmp[:],
                                         in1=bc2s[:])
        orx_es.close()
        zb_es.close()

        if upto < 5:
            _early_out(nc, tc, y_out)
            h2b_es.close()
            _split_multiwaits(nc)
            return nc

        # ============ Phase 5a: MLP up-proj + gelu ========================
        g_es = ExitStack()
        g_pool = g_es.enter_context(tc.tile_pool(name="gp", bufs=1))
        gt = g_pool.tile([128, NF1, 512], bf16, name="gt")
        with ExitStack() as ph6:
            w1p = ph6.enter_context(tc.tile_pool(name="w1p", bufs=2))
            ps_m = ph6.enter_context(tc.tile_pool(name="ps_m",
                                                  bufs=2, space="PSUM"))
            for g in range(16):
                wch = w1p.tile([128, NDT, 512], bf16, name="w1c")
                nc.sync.dma_start(
                    out=wch[:],
                    in_=w1src[:, :, g * 512:(g + 1) * 512])
                accs = [ps_m.tile([128, 512], f32, name=f"ma{ff}")
                        for ff in range(4)]
                for t in range(NDT):
                    for ff in range(4):
                        nc.tensor.matmul(
                            accs[ff][:],
                            wch[:, t, ff * 128:(ff + 1) * 128],
                            h2Tb[:, t, :],
                            start=(t == 0), stop=(t == NDT - 1))
                for ff in range(4):
                    f1 = g * 4 + ff
                    nc.scalar.activation(
                        out=gt[:, f1, :], in_=accs[ff][:],
                        func=AF.Gelu_apprx_tanh,
                        bias=b1_c[:, f1:f1 + 1], scale=1.0)
        if upto < 6:
            _early_out(nc, tc, y_out)
            g_es.close()
            h2b_es.close()
            _split_multiwaits(nc)
            return nc

        # ============ Phase 5b: MLP down-proj + residual + store ==========
        with ExitStack() as ph7:
            w2p = ph7.enter_context(tc.tile_pool(name="w2p", bufs=2))
            ost = ph7.enter_context(tc.tile_pool(name="ost", bufs=2))
            ys = ph7.enter_context(tc.tile_pool(name="ys", bufs=2))
            ps_m2 = ph7.enter_context(tc.tile_pool(name="ps_m2",
                                                   bufs=1, space="PSUM"))
            ps_tp3 = ph7.enter_context(tc.tile_pool(name="ps_tp3",
                                                    bufs=2, space="PSUM"))
            for dg in range(4):
                accs = [ps_m2.tile([128, 512], f32, name=f"mb{dd}")
                        for dd in range(4)]
                for quar in range(4):
                    wch = w2p.tile([128, 16, 512], bf16, name="w2c")
                    nc.sync.dma_start(
                        out=wch[:],
                        in_=w2src[:, quar * 16:(quar + 1) * 16,
                                  dg * 512:(dg + 1) * 512])
                    for fi in range(16):
                        ft = quar * 16 + fi
                        for dd in range(4):
                            nc.tensor.matmul(
                                accs[dd][:],
                                wch[:, fi, dd * 128:(dd + 1) * 128],
                                gt[:, ft, :],
                                start=(ft == 0), stop=(ft == NF1 - 1))
                outg = ost.tile([128, 4, 512], f32, name="outg")
                for dd in range(4):
                    dt = dg * 4 + dd
                    mb = ost.tile([128, 512], f32, name="mbb")
                    nc.scalar.activation(out=mb[:], in_=accs[dd][:],
                                         func=AF.Identity,
                                         bias=b2_c[:, dt:dt + 1], scale=1.0)
                    # out = z2*g2 + (m + b2')
                    nc.vector.scalar_tensor_tensor(
                        out=outg[:, dd, :], in0=h2Tb[:, dt, :],
                        scalar=g2_c[:, dt:dt + 1], in1=mb[:],
                        op0=ALU.mult, op1=ALU.add)
                ystage = ys.tile([128, 4, 512], f32, name="ystage")
                for dd in range(4):
                    for rseg in range(4):
                        tp = ps_tp3.tile([128, 128], f32, name="tpo")
                        nc.tensor.transpose(
                            tp[:],
                            outg[:, dd, rseg * 128:(rseg + 1) * 128],
                            ident32[:])
                        nc.vector.tensor_copy(
                            out=ystage[:, rseg, dd * 128:(dd + 1) * 128],
                            in_=tp[:])
                for rseg in range(4):
                    nc.sync.dma_start(
                        out=y_out[rseg * 128:(rseg + 1) * 128,
                                  dg * 512:(dg + 1) * 512],
                        in_=ystage[:, rseg, :])
        g_es.close()
        h2b_es.close()

    _split_multiwaits(nc)
    return nc


def _early_out(nc, tc, y_out):
    with ExitStack() as es:
        p = es.enter_context(tc.tile_pool(name="eo", bufs=1))
        z = p.tile([128, 16], f32, name="eoz")
        nc.vector.memset(z[:], 0.0)
        nc.sync.dma_start(out=y_out[:128, :16], in_=z[:])


def _split_multiwaits(nc, max_waits=1):
    """walrus in this toolchain rejects >1 sem-wait on most instruction
    structs; split extras onto preceding sequencer NoOps (same engine)."""
    if getattr(nc, "_skip_split_multiwaits", False):
        return
    for fn in nc.m.functions:
        for bb in fn.blocks:
            new_list, changed = [], False
            for inst in bb.instructions:
                si = inst.sync_info
                lim = max_waits
                if si is not None and len(si.on_wait) > lim:
                    waits = list(si.on_wait)
                    for k, w in enumerate(waits[:-lim]):
                        nop = mybir.InstNoOp(name=f"{inst.name}-splitw{k}")
                        nop.engine = inst.engine
                        nop.sync_info = mybir.SyncInfo(on_wait=[w],
                                                       on_update=[])
                        new_list.append(nop)
                    inst.sync_info = mybir.SyncInfo(
                        on_wait=waits[-lim:],
                        on_update=list(si.on_update))
                    changed = True
                new_list.append(inst)
            if changed:
                bb.instructions = new_list


# ---------------------------------------------------------------------------
# Persistent SPMD runner (compile once per process, reuse executable).

class SpmdKernel:
    def __init__(self, nc, n_cores):
        import jax
        from jax.sharding import Mesh, PartitionSpec
        from jax.experimental.shard_map import shard_map
        from concourse.bass2jax import (_bass_exec_p, install_neuronx_cc_hook,
                                        partition_id_tensor)
        self.jax = jax
        self.PartitionSpec = PartitionSpec
        install_neuronx_cc_hook()
        self.nc = nc
        self.n_cores = n_cores
        partition_name = (nc.partition_id_tensor.name
                          if nc.partition_id_tensor else None)
        in_names, out_names, out_avals, zero_outs = [], [], [], []
        for alloc in nc.m.functions[0].allocations:
            if not isinstance(alloc, mybir.MemoryLocationSet):
                continue
            name = alloc.memorylocations[0].name
            if alloc.kind == "ExternalInput":
                if name != partition_name:
                    in_names.append(name)
            elif alloc.kind == "ExternalOutput":
                shape = tuple(alloc.tensor_shape)
                dtype = mybir.dt.np(alloc.dtype)
                out_names.append(name)
                out_avals.append(jax.core.ShapedArray(shape, dtype))
                zero_outs.append(np.zeros(shape, dtype))
        n_params = len(in_names)
        n_outs = len(out_avals)
        all_in_names = list(in_names) + list(out_names)
        if partition_name is not None:
            all_in_names.append(partition_name)
        self.in_names = in_names
        self.out_names = out_names
        self.out_avals = out_avals
        self.zero_outs = zero_outs
        self.n_params = n_params

        def _body(*args):
            operands = list(args)
            if partition_name is not None:
                operands.append(partition_id_tensor())
            outs = _bass_exec_p.bind(
                *operands,
                out_avals=tuple(out_avals),
                in_names=tuple(all_in_names),
                out_names=tuple(out_names),
                lowering_input_output_aliases=(),
                sim_require_finite=True,
                sim_require_nnan=True,
                nc=nc,
            )
            return tuple(outs)

        devices = jax.devices()[:n_cores]
        assert len(devices) == n_cores
        self.mesh = Mesh(np.asarray(devices), ("core",))
        in_specs = (PartitionSpec("core"),) * (n_params + n_outs)
        out_specs = (PartitionSpec("core"),) * n_outs
        self.fn = jax.jit(
            shard_map(_body, mesh=self.mesh, in_specs=in_specs,
                      out_specs=out_specs, check_rep=False),
            keep_unused=True,
        )

    def stage_inputs(self, in_maps):
        from jax.sharding import NamedSharding
        per_core = [[np.asarray(m[name]) for name in self.in_names]
                    for m in in_maps]
        concat_in = [
            np.ascontiguousarray(np.concatenate(
                [per_core[c][i] for c in range(self.n_cores)], axis=0))
            for i in range(self.n_params)
        ]
        concat_zeros = [
            np.zeros((self.n_cores * z.shape[0], *z.shape[1:]), z.dtype)
            for z in self.zero_outs
        ]
        sh = NamedSharding(self.mesh, self.PartitionSpec("core"))
        return [self.jax.device_put(a, sh) for a in (concat_in + concat_zeros)]

    def run_staged(self, args):
        outs = self.fn(*args)
        self.jax.block_until_ready(outs)
        return outs

    def results(self, outs):
        res = []
        for c in range(self.n_cores):
            res.append({
                name: np.asarray(outs[i]).reshape(
                    self.n_cores, *self.out_avals[i].shape)[c]
                for i, name in enumerate(self.out_names)
            })
        return res

    def __call__(self, in_maps):
        return self.results(self.run_staged(self.stage_inputs(in_maps)))


_NC_CACHE = {}


def get_runner(upto=9, nocc=False):
    key = f"runner-{upto}-{nocc}"
    if key not in _NC_CACHE:
        nc = build_nc(upto, nocc)
        _NC_CACHE[key] = SpmdKernel(nc, N_CORES)
    return _NC_CACHE[key]


def host_prep(inputs):
    import ml_dtypes
    bf = ml_dtypes.bfloat16

    def a32(v):
        return np.asarray(v, np.float32)
    x = a32(inputs["x"])
    Wq, Wk, Wv = a32(inputs["Wq"]), a32(inputs["Wk"]), a32(inputs["Wv"])
    Wo = a32(inputs["Wo"])
    W1, W2 = a32(inputs["W1"]), a32(inputs["W2"])
    g1, be1 = a32(inputs["ln1_g"]), a32(inputs["ln1_b"])
    g2, be2 = a32(inputs["ln2_g"]), a32(inputs["ln2_b"])
    x_flat = np.ascontiguousarray(x.reshape(NROW, D))
    # fold LN1 affine into Wqkv: W' = W * g1 (input-feature scale),
    # b' = b + W @ beta1
    wqkvT = np.ascontiguousarray(np.concatenate(
        [(Wk * g1[None, :]).T, (Wq * g1[None, :]).T,
         (Wv * g1[None, :]).T], axis=1).astype(bf))
    bqkv = np.concatenate([a32(inputs["bk"]) + Wk @ be1,
                           a32(inputs["bq"]) + Wq @ be1,
                           a32(inputs["bv"]) + Wv @ be1])
    woT = np.ascontiguousarray(Wo.T.astype(bf))  # [in-feature, dout]
    bo = a32(inputs["bo"]) + be1
    w1T = np.ascontiguousarray((W1 * g2[None, :]).T.astype(bf))
    b1 = a32(inputs["b1"]) + W1 @ be2
    w2T = np.ascontiguousarray(W2.T.astype(bf))
    b2 = a32(inputs["b2"]) + be2
    shared = {
        "wqkvT": wqkvT, "bqkv": bqkv, "woT": woT, "w1T": w1T, "w2T": w2T,
        "bo": bo, "b1": b1, "b2": b2, "g1": g1, "g2": g2,
    }
    in_maps = []
    for c in range(N_CORES):
        m = dict(shared)
        m["x"] = np.ascontiguousarray(x_flat[ROWS * c: ROWS * (c + 1)])
        in_maps.append(m)
    return in_maps


def kernel(**inputs) -> np.ndarray:
    in_maps = host_prep(inputs)
    runner = get_runner()
    res = runner(in_maps)
    out = np.concatenate([res[c]["y"] for c in range(N_CORES)], axis=0)
    return out.reshape(B, S, D)
